# revision 36
# baseline (speedup 1.0000x reference)
"""Biased attention Trainium2 kernel, SPMD over 8 NeuronCores.

Problem (per reference):
    sim  = q @ k^T / sqrt(64)                       [b,h,i,j]
    sim  = where(mask[b,j], sim, -fmax)
    sim -= taus[h] * attn_bias[b,i,j]
    out  = softmax(sim, axis=j) @ v                 [b,h,i,d]

Shapes: B=2, H=16, S=2048, D=64, fp32.

Sharding: batch*heads across 8 cores -> 4 heads per core, all with the
same batch b (core c handles b=c//4, heads 4*(c%4)..4*(c%4)+3), so
attn_bias/mask are batch-sharded and loaded once per core.

Key idea vs a dense kernel: the key-padding mask is known on the host, so
masked-out keys (~half of them) are GATHERED AWAY on the host.  Each core
receives only the valid keys (padded up to a multiple of 128), which cuts
the QK matmul, the bias application, the softmax exp and the attn@V matmul
proportionally.  Padded key rows get K=0 / V=0 / bias=0 and a -1e30
per-partition addend folded into the exp activation, so they contribute
exp(-1e30)=0 and drop out of the softmax exactly like masked keys.

Per-core dataflow (all on device):
  - scores are computed TRANSPOSED: zT[j,i] = K Q^T, j on partitions, so
    softmax reductions run along the matmul contraction instead of needing
    a big transpose of the attention matrix.  Head pairs are packed on
    partitions (even head rows 0-63, odd head 64-127).
  - the tau*attn_bias subtraction runs on the DVE as a fused
    (biasT * -8tau) + scores op into SBUF (measured much faster on HW
    than the PE scaled-identity alternative, which pays exposed weight
    loads; the BIAS_SCHED string can still steer individual j-tiles to
    the PE).  1/sqrt(d)=1/8 folds into the exp scale.
  - q/k/bias are loaded in bf16 (half the HBM traffic, fast PE weight
    loads); all operands are DMA'd directly in their matmul dtype (no
    conversion copies on device).  V stays f32r.
  - V gets a ones-column appended (on host) so the softmax denominator
    falls out of the attention @ V matmul for free.
  - output is stored TRANSPOSED and UNNORMALIZED ([d,i] plus a denominator
    row); the host divides and transposes while unsharding, which is O(out)
    work and keeps the device drain to a single PSUM->SBUF copy + DMA.
  - the two head-pair chains interleave in the inner loop (two
    QK->stt->exp->AV chains in flight over 2 PSUM score buffers); input
    tiles are double-buffered so back-to-back invocations overlap; V and
    output DMAs ride the idle GPSIMD queue.
"""

import math
import numpy as np
from contextlib import ExitStack

import ml_dtypes

import concourse.bass as bass
import concourse.tile as tile
from concourse import bacc, mybir
from concourse import bass_utils

F32 = mybir.dt.float32
F32R = mybir.dt.float32r
BF16 = mybir.dt.bfloat16
Alu = mybir.AluOpType
Act = mybir.ActivationFunctionType

B, H, S, D = 2, 16, 2048, 64
N_CORES = 8
HPC = 4          # heads per core
NP = S // 512    # 4 i-panels of 512
BIG = 1.0e30

# per-j-tile engine for the tau*bias application: 'd' -> DVE stt,
# 'p' -> PE scaled-identity matmul.  Tuned so PE/DVE/ACT busy times are
# balanced.  Extended/truncated cyclically to the actual j-tile count.
BIAS_SCHED = "ddddddddd"
# j-offset between the two head-pair chains (anti-aligns engine bursts)
STAGGER = 2
# dtype for the q/k matmul operands. BF16 enables the PE fast-weight-load
# path (f32r weights load at half rate) and halves the q/k DMA; the score
# error it introduces (~4e-3 relative) is well inside the 2e-2 gate.
QK_DT = mybir.dt.bfloat16
# fuse the two head-pairs' softmax exp into one FD=2048 ACT instruction
# (only valid when the whole schedule is 'd'); saves the per-instruction
# ACT init cost on the bottleneck engine.
# fusing both head-pairs' exp into one FD=2048 ACT measured SLOWER on HW
# (ties up both PSUM score buffers per slot -> no pipeline lookahead).
EXP_PAIR = False
# pass explicit tile_position on the QK matmuls (distinct PE row groups)
QK_TILE_POS = True


def _build(njt, sched=BIAS_SCHED, n_rep=1, num_devices=N_CORES):
    nj = njt * 128
    nc = bacc.Bacc("TRN2", target_bir_lowering=False, debug=False,
                   num_devices=num_devices)

    q_ap = nc.dram_tensor("qt", [2, 128, S], QK_DT, kind="ExternalInput").ap()
    k_ap = nc.dram_tensor("kt", [2, 128, nj], QK_DT,
                          kind="ExternalInput").ap()
    v_ap = nc.dram_tensor("vp", [HPC, 128, njt * 65], F32R,
                          kind="ExternalInput").ap()
    n8tau_ap = nc.dram_tensor("n8tau", [128, HPC], F32,
                              kind="ExternalInput").ap()
    si_ap = nc.dram_tensor("scaledI", [128, HPC * 128], BF16,
                           kind="ExternalInput").ap()
    ma_ap = nc.dram_tensor("maskadd", [128, njt], F32,
                           kind="ExternalInput").ap()
    bias_ap = nc.dram_tensor("biasT", [NP, 128, njt, 512], BF16,
                             kind="ExternalInput").ap()
    out_ap = nc.dram_tensor("out", [HPC, NP, 65, 512], F32,
                            kind="ExternalOutput").ap()

    with tile.TileContext(nc) as tc:
        with ExitStack() as ctx:
            _body(ctx, tc, njt, sched, n_rep, q_ap, k_ap, v_ap, n8tau_ap,
                  si_ap, ma_ap, bias_ap, out_ap)

    nc.compile()
    return nc


def _body(ctx, tc, njt, sched, n_rep, q_ap, k_ap, v_ap, n8tau_ap, si_ap,
          ma_ap, bias_ap, out_ap):
    nc = tc.nc
    nj = njt * 128
    eng = [sched[j % len(sched)] for j in range(njt)]

    # all input tiles double-buffered so the next rep's (or next panel's)
    # loads overlap this rep's compute; V / small loads and the output
    # stores go through the otherwise-idle GPSIMD (Pool) DMA queue so the
    # SP queue only carries q/k/bias.
    inp = ctx.enter_context(tc.tile_pool(name="inp", bufs=2))
    bpool = ctx.enter_context(tc.tile_pool(name="bpool", bufs=2))
    epool = ctx.enter_context(tc.tile_pool(name="epool", bufs=8))
    spool = ctx.enter_context(tc.tile_pool(name="spool", bufs=8))
    dpool = ctx.enter_context(tc.tile_pool(name="dpool", bufs=4))
    zps = ctx.enter_context(tc.tile_pool(name="zps", bufs=2, space="PSUM"))
    ops = ctx.enter_context(tc.tile_pool(name="ops", bufs=1, space="PSUM"))

    for _rep in range(n_rep):
        _rep_body(tc, njt, eng, inp, bpool, epool, spool, dpool, zps, ops,
                  q_ap, k_ap, v_ap, n8tau_ap, si_ap, ma_ap, bias_ap, out_ap)


def _rep_body(tc, njt, eng, inp, bpool, epool, spool, dpool, zps, ops,
              q_ap, k_ap, v_ap, n8tau_ap, si_ap, ma_ap, bias_ap, out_ap):
    nc = tc.nc
    nj = njt * 128

    # ---- constants (host-prepared, DMA'd in final dtype) ---------------
    n8tau = inp.tile([128, HPC], F32, tag="n8tau")
    nc.gpsimd.dma_start(n8tau[:], n8tau_ap[:])
    scaledI = inp.tile([128, HPC * 128], BF16, tag="scaledI")
    nc.gpsimd.dma_start(scaledI[:], si_ap[:])
    maskadd = inp.tile([128, njt], F32, tag="maskadd")
    nc.gpsimd.dma_start(maskadd[:], ma_ap[:])

    # Q^T / K^T head-pair tiles: even head on partitions 0-63, odd head on
    # 64-127 (concurrent row-group QK matmuls).
    qtr = []
    ktr = []
    for pair in range(2):
        qt = inp.tile([128, S], QK_DT, tag=f"qtr{pair}")
        nc.sync.dma_start(qt[:], q_ap[pair])
        qtr.append(qt)
        kt = inp.tile([128, nj], QK_DT, tag=f"ktr{pair}")
        nc.sync.dma_start(kt[:], k_ap[pair])
        ktr.append(kt)

    # V with ones column, host-premarshalled [128, njt*65] per head
    vaug = []
    for h in range(HPC):
        va = inp.tile([128, njt * 65], F32R, tag=f"vaug{h}")
        nc.gpsimd.dma_start(va[:], v_ap[h])
        vaug.append(va)

    # ---- main loops ----------------------------------------------------
    # The two head-pair chains are interleaved in the inner loop so two
    # (QK -> bias -> exp -> AV) dependency chains are always in flight;
    # the DVE bias path writes to SBUF so the scores PSUM tile is freed as
    # soon as the stt has read it (shorter PSUM lifetime -> better overlap
    # with only 2 score buffers).
    for P in range(NP):
        isl = slice(P * 512, (P + 1) * 512)
        # per-panel bias tile (bf16), double-buffered against next panel
        biasP = bpool.tile([128, njt * 512], BF16, tag="biasP",
                           name=f"biasP_{P}")
        bview = biasP[:].rearrange("p (j x) -> p j x", j=njt)
        nc.sync.dma_start(bview[:], bias_ap[P])
        o = [[ops.tile([128, 512], F32, tag=f"o{pair}{t}",
                       name=f"o{pair}{t}_{P}")
              for t in range(2)] for pair in range(2)]
        # pair 1 walks the j-tiles at an offset so its PE/DVE bias slots
        # anti-align with pair 0's (accumulation order over j is free).
        stag = STAGGER if njt > 3 else 0
        exp_pair = EXP_PAIR and all(e == 'd' for e in eng)
        if exp_pair:
            stag = 0
        for jj in range(njt):
            zs2 = None
            if exp_pair:
                zs2 = spool.tile([128, 2048], F32, tag="zs2",
                                 name=f"zs2_{P}_{jj}")
            for pair in range(2):
                j = (jj + stag * pair) % njt
                jsl = slice(j * 128, (j + 1) * 128)
                bT = bview[:, j]
                use_pe = (eng[j] == 'p')
                zp = zps.tile([128, 1024], F32, tag="zp",
                              name=f"zp_{P}_{pair}_{j}")
                for t in range(2):
                    psl = slice(t * 64, (t + 1) * 64)
                    zsl = slice(t * 512, (t + 1) * 512)
                    nc.tensor.matmul(zp[:, zsl], lhsT=ktr[pair][psl, jsl],
                                     rhs=qtr[pair][psl, isl],
                                     start=True, stop=not use_pe,
                                     tile_position=((t * 64, 0)
                                                    if QK_TILE_POS else None))
                if use_pe:
                    for t in range(2):
                        h = 2 * pair + t
                        zsl = slice(t * 512, (t + 1) * 512)
                        nc.tensor.matmul(
                            zp[:, zsl],
                            lhsT=scaledI[:, h * 128:(h + 1) * 128],
                            rhs=bT, start=False, stop=True,
                            skip_group_check=True)
                    esrc = zp
                else:
                    zs = (zs2[:, pair * 1024:(pair + 1) * 1024] if exp_pair
                          else spool.tile([128, 1024], F32, tag="zs",
                                          name=f"zs_{P}_{pair}_{j}")[:])
                    for t in range(2):
                        h = 2 * pair + t
                        zsl = slice(t * 512, (t + 1) * 512)
                        nc.vector.scalar_tensor_tensor(
                            zs[:, zsl], in0=bT, scalar=n8tau[:, h:h + 1],
                            in1=zp[:, zsl], op0=Alu.mult, op1=Alu.add)
                    esrc = zs
                if exp_pair:
                    continue   # one fused exp + the AVs are emitted below
                et = epool.tile([128, 1024], F32R, tag="et",
                                name=f"et_{P}_{pair}_{j}")
                nc.scalar.activation(et[:], esrc[:], Act.Exp,
                                     bias=maskadd[:, j:j + 1], scale=0.125)
                for t in range(2):
                    h = 2 * pair + t
                    nc.tensor.matmul(
                        o[pair][t][0:65, :],
                        lhsT=vaug[h][:, j * 65:(j + 1) * 65],
                        rhs=et[:, t * 512:(t + 1) * 512],
                        start=(jj == 0), stop=(jj == njt - 1))
            if exp_pair:
                j = jj
                et = epool.tile([128, 2048], F32R, tag="et2",
                                name=f"et2_{P}_{jj}")
                nc.scalar.activation(et[:], zs2[:], Act.Exp,
                                     bias=maskadd[:, j:j + 1], scale=0.125)
                for pair in range(2):
                    for t in range(2):
                        h = 2 * pair + t
                        nc.tensor.matmul(
                            o[pair][t][0:65, :],
                            lhsT=vaug[h][:, j * 65:(j + 1) * 65],
                            rhs=et[:, (2 * pair + t) * 512:
                                    (2 * pair + t + 1) * 512],
                            start=(jj == 0), stop=(jj == njt - 1))

        # ---- drain: PSUM->SBUF copy + DMA (normalize on host) ----------
        for pair in range(2):
            for t in range(2):
                h = 2 * pair + t
                ob = dpool.tile([65, 512], F32, tag="ob", name=f"ob_{P}_{h}")
                nc.vector.tensor_copy(ob[:], o[pair][t][0:65, :])
                nc.gpsimd.dma_start(out_ap[h, P], ob[:])


_NC_CACHE = {}


def _get_nc(njt, sched=BIAS_SCHED, n_rep=1):
    key = (njt, sched, n_rep)
    if key not in _NC_CACHE:
        _NC_CACHE[key] = _build(njt, sched, n_rep)
    return _NC_CACHE[key]


def prepare_in_maps(q, k, v, mask, taus, attn_bias):
    """Host-side shard + gather. Returns (in_maps, njt, per-core key counts)."""
    q = np.asarray(q, dtype=np.float32)
    k = np.asarray(k, dtype=np.float32)
    v = np.asarray(v, dtype=np.float32)
    mask = np.asarray(mask).astype(bool)
    taus = np.asarray(taus, dtype=np.float32)
    attn_bias = np.asarray(attn_bias, dtype=np.float32)

    idxs = [np.nonzero(mask[b])[0] for b in range(B)]
    njs = [len(ix) for ix in idxs]
    njt = max(1, (max(njs) + 127) // 128)
    nj = njt * 128

    eye = np.eye(128, dtype=np.float32)

    per_batch = []
    for b in range(B):
        ix = idxs[b]
        n = njs[b]
        # gathered K^T [H,D,nj], V-aug [H,128,njt*65], bias [NP,128,njt,512]
        kg = np.zeros((H, D, nj), dtype=np.float32)
        kg[:, :, :n] = k[b][:, ix, :].swapaxes(1, 2)
        vg = np.zeros((H, nj, D), dtype=np.float32)
        vg[:, :n, :] = v[b][:, ix, :]
        va = np.ones((H, 128, njt, 65), dtype=np.float32)
        va[..., 0:64] = vg.reshape(H, njt, 128, D).transpose(0, 2, 1, 3)
        va = np.ascontiguousarray(va.reshape(H, 128, njt * 65))
        bg = np.zeros((nj, S), dtype=np.float32)
        bg[:n] = attn_bias[b][:, ix].T
        # [P, p, j, x] = biasT[j*128+p, P*512+x]
        bp = np.ascontiguousarray(
            bg.reshape(njt, 128, NP, 512).transpose(2, 1, 0, 3)
        ).astype(ml_dtypes.bfloat16)
        ma = np.zeros((128, njt), dtype=np.float32)
        g = np.arange(nj).reshape(njt, 128).T  # [p, j] global key index
        ma[g >= n] = -BIG
        per_batch.append((kg, va, bp, ma))

    in_maps = []
    for c in range(N_CORES):
        b = c // 4
        h0 = (c % 4) * 4
        kg, va, bp, ma = per_batch[b]
        t4 = taus[h0:h0 + HPC]
        si = np.einsum('pc,h->phc', eye, -8.0 * t4).reshape(128, HPC * 128)
        qk_np = (ml_dtypes.bfloat16 if QK_DT == mybir.dt.bfloat16
                 else np.float32)
        in_maps.append({
            "qt": np.ascontiguousarray(
                q[b, h0:h0 + HPC].swapaxes(1, 2)).reshape(2, 128, S)
                .astype(qk_np),
            "kt": np.ascontiguousarray(
                kg[h0:h0 + HPC]).reshape(2, 128, -1).astype(qk_np),
            "vp": va[h0:h0 + HPC],
            "n8tau": np.broadcast_to((-8.0 * t4)[None, :], (128, HPC)).copy(),
            "scaledI": si.astype(ml_dtypes.bfloat16),
            "maskadd": ma,
            "biasT": bp,
        })
    return in_maps, njt, njs


def kernel(q, k, v, mask, taus, attn_bias):
    in_maps, njt, _ = prepare_in_maps(q, k, v, mask, taus, attn_bias)
    nc = _get_nc(njt)
    res = bass_utils.run_bass_kernel_spmd(nc, in_maps,
                                          core_ids=list(range(N_CORES)))
    out = np.empty((B, H, S, D), dtype=np.float32)
    for c in range(N_CORES):
        b = c // 4
        h0 = (c % 4) * 4
        arr = res.results[c]["out"]          # [HPC, NP, 65, 512]
        norm = arr[:, :, 0:64, :] / arr[:, :, 64:65, :]
        # i = P*512 + col  ->  out[h, i, d]
        out[b, h0:h0 + HPC] = norm.transpose(0, 1, 3, 2).reshape(HPC, S, D)
    return out


if __name__ == "__main__":
    rng = np.random.default_rng(0)
    inputs = {
        "q": rng.standard_normal((B, H, S, D), dtype=np.float32),
        "k": rng.standard_normal((B, H, S, D), dtype=np.float32),
        "v": rng.standard_normal((B, H, S, D), dtype=np.float32),
        "mask": rng.random((B, S)) < 0.5,
        "taus": rng.random(H, dtype=np.float32),
        "attn_bias": rng.random((B, S, S), dtype=np.float32),
    }
    o = kernel(**inputs)
    print("out", o.shape, o.dtype, np.isfinite(o).all())


# revision 38
# speedup vs baseline: 1.1036x; 1.1036x over previous
"""Biased attention Trainium2 kernel, SPMD over 8 NeuronCores.

Problem (per reference):
    sim  = q @ k^T / sqrt(64)                       [b,h,i,j]
    sim  = where(mask[b,j], sim, -fmax)
    sim -= taus[h] * attn_bias[b,i,j]
    out  = softmax(sim, axis=j) @ v                 [b,h,i,d]

Shapes: B=2, H=16, S=2048, D=64, fp32.

Sharding: batch*heads across 8 cores -> 4 heads per core, all with the
same batch b (core c handles b=c//4, heads 4*(c%4)..4*(c%4)+3), so
attn_bias/mask are batch-sharded and loaded once per core.

Key idea vs a dense kernel: the key-padding mask is known on the host, so
masked-out keys (~half of them) are GATHERED AWAY on the host.  Each core
receives only the valid keys (padded up to a multiple of 128), which cuts
the QK matmul, the bias application, the softmax exp and the attn@V matmul
proportionally.  Padded key rows get K=0 / V=0 / bias=0 and a -1e30
per-partition addend folded into the exp activation, so they contribute
exp(-1e30)=0 and drop out of the softmax exactly like masked keys.

Per-core dataflow (all on device):
  - scores are computed TRANSPOSED: zT[j,i] = K Q^T, j on partitions, so
    softmax reductions run along the matmul contraction instead of needing
    a big transpose of the attention matrix.  Head pairs are packed on
    partitions (even head rows 0-63, odd head 64-127).
  - the tau*attn_bias subtraction runs on the DVE as a fused
    (biasT * -8tau) + scores op into SBUF (measured much faster on HW
    than the PE scaled-identity alternative, which pays exposed weight
    loads; the BIAS_SCHED string can still steer individual j-tiles to
    the PE).  1/sqrt(d)=1/8 folds into the exp scale.
  - q/k/bias are loaded in bf16 (half the HBM traffic, fast PE weight
    loads); all operands are DMA'd directly in their matmul dtype (no
    conversion copies on device).  V stays f32r.
  - V gets a ones-column appended (on host) so the softmax denominator
    falls out of the attention @ V matmul for free.
  - output is stored TRANSPOSED and UNNORMALIZED ([d,i] plus a denominator
    row); the host divides and transposes while unsharding, which is O(out)
    work and keeps the device drain to a single PSUM->SBUF copy + DMA.
  - the two head-pair chains interleave in the inner loop (two
    QK->stt->exp->AV chains in flight over 2 PSUM score buffers); input
    tiles are double-buffered so back-to-back invocations overlap; V and
    output DMAs ride the idle GPSIMD queue.
"""

import math
import numpy as np
from contextlib import ExitStack

import ml_dtypes

import concourse.bass as bass
import concourse.tile as tile
from concourse import bacc, mybir
from concourse import bass_utils

F32 = mybir.dt.float32
F32R = mybir.dt.float32r
BF16 = mybir.dt.bfloat16
Alu = mybir.AluOpType
Act = mybir.ActivationFunctionType

B, H, S, D = 2, 16, 2048, 64
N_CORES = 8
HPC = 4          # heads per core
NP = S // 512    # 4 i-panels of 512
BIG = 1.0e30

# per-j-tile engine for the tau*bias application: 'd' -> DVE stt,
# 'p' -> PE scaled-identity matmul.  Tuned so PE/DVE/ACT busy times are
# balanced.  Extended/truncated cyclically to the actual j-tile count.
BIAS_SCHED = "ddddddddd"
# j-offset between the two head-pair chains (anti-aligns engine bursts)
STAGGER = 2
# dtype for the q/k matmul operands. BF16 enables the PE fast-weight-load
# path (f32r weights load at half rate) and halves the q/k DMA; the score
# error it introduces (~4e-3 relative) is well inside the 2e-2 gate.
QK_DT = mybir.dt.bfloat16
# fuse the two head-pairs' softmax exp into one FD=2048 ACT instruction
# (only valid when the whole schedule is 'd'); saves the per-instruction
# ACT init cost on the bottleneck engine.
# fusing both head-pairs' exp into one FD=2048 ACT measured SLOWER on HW
# (ties up both PSUM score buffers per slot -> no pipeline lookahead).
EXP_PAIR = False
# pass explicit tile_position on the QK matmuls (distinct PE row groups)
QK_TILE_POS = True
# route V/out/small DMAs through the GPSIMD (Pool) software-DGE queue to
# offload the SP queue; False puts everything on SP (hardware DGE).
POOL_DMA = True


def _build(njt, sched=BIAS_SCHED, n_rep=1, num_devices=N_CORES):
    nj = njt * 128
    nc = bacc.Bacc("TRN2", target_bir_lowering=False, debug=False,
                   num_devices=num_devices)

    q_ap = nc.dram_tensor("qt", [2, 128, S], QK_DT, kind="ExternalInput").ap()
    k_ap = nc.dram_tensor("kt", [2, 128, nj], QK_DT,
                          kind="ExternalInput").ap()
    v_ap = nc.dram_tensor("vp", [HPC, 128, njt * 65], F32R,
                          kind="ExternalInput").ap()
    n8tau_ap = nc.dram_tensor("n8tau", [128, HPC], F32,
                              kind="ExternalInput").ap()
    si_ap = nc.dram_tensor("scaledI", [128, HPC * 128], BF16,
                           kind="ExternalInput").ap()
    ma_ap = nc.dram_tensor("maskadd", [128, njt], F32,
                           kind="ExternalInput").ap()
    bias_ap = nc.dram_tensor("biasT", [NP, 128, njt, 512], BF16,
                             kind="ExternalInput").ap()
    out_ap = nc.dram_tensor("out", [HPC, NP, 65, 512], F32,
                            kind="ExternalOutput").ap()

    with tile.TileContext(nc) as tc:
        with ExitStack() as ctx:
            _body(ctx, tc, njt, sched, n_rep, q_ap, k_ap, v_ap, n8tau_ap,
                  si_ap, ma_ap, bias_ap, out_ap)

    nc.compile()
    return nc


def _body(ctx, tc, njt, sched, n_rep, q_ap, k_ap, v_ap, n8tau_ap, si_ap,
          ma_ap, bias_ap, out_ap):
    nc = tc.nc
    nj = njt * 128
    eng = [sched[j % len(sched)] for j in range(njt)]

    # all input tiles double-buffered so the next rep's (or next panel's)
    # loads overlap this rep's compute; V / small loads and the output
    # stores go through the otherwise-idle GPSIMD (Pool) DMA queue so the
    # SP queue only carries q/k/bias.
    inp = ctx.enter_context(tc.tile_pool(name="inp", bufs=2))
    bpool = ctx.enter_context(tc.tile_pool(name="bpool", bufs=2))
    epool = ctx.enter_context(tc.tile_pool(name="epool", bufs=8))
    spool = ctx.enter_context(tc.tile_pool(name="spool", bufs=8))
    dpool = ctx.enter_context(tc.tile_pool(name="dpool", bufs=4))
    zps = ctx.enter_context(tc.tile_pool(name="zps", bufs=2, space="PSUM"))
    ops = ctx.enter_context(tc.tile_pool(name="ops", bufs=1, space="PSUM"))

    for _rep in range(n_rep):
        _rep_body(tc, njt, eng, inp, bpool, epool, spool, dpool, zps, ops,
                  q_ap, k_ap, v_ap, n8tau_ap, si_ap, ma_ap, bias_ap, out_ap)


def _rep_body(tc, njt, eng, inp, bpool, epool, spool, dpool, zps, ops,
              q_ap, k_ap, v_ap, n8tau_ap, si_ap, ma_ap, bias_ap, out_ap):
    nc = tc.nc
    nj = njt * 128
    aux = nc.gpsimd if POOL_DMA else nc.sync

    # ---- constants (host-prepared, DMA'd in final dtype) ---------------
    n8tau = inp.tile([128, HPC], F32, tag="n8tau")
    aux.dma_start(n8tau[:], n8tau_ap[:])
    scaledI = inp.tile([128, HPC * 128], BF16, tag="scaledI")
    aux.dma_start(scaledI[:], si_ap[:])
    maskadd = inp.tile([128, njt], F32, tag="maskadd")
    aux.dma_start(maskadd[:], ma_ap[:])

    # Q^T / K^T head-pair tiles: even head on partitions 0-63, odd head on
    # 64-127 (concurrent row-group QK matmuls).
    qtr = []
    ktr = []
    for pair in range(2):
        qt = inp.tile([128, S], QK_DT, tag=f"qtr{pair}")
        nc.sync.dma_start(qt[:], q_ap[pair])
        qtr.append(qt)
        kt = inp.tile([128, nj], QK_DT, tag=f"ktr{pair}")
        nc.sync.dma_start(kt[:], k_ap[pair])
        ktr.append(kt)

    # V with ones column, host-premarshalled [128, njt*65] per head
    vaug = []
    for h in range(HPC):
        va = inp.tile([128, njt * 65], F32R, tag=f"vaug{h}")
        aux.dma_start(va[:], v_ap[h])
        vaug.append(va)

    # ---- main loops ----------------------------------------------------
    # The two head-pair chains are interleaved in the inner loop so two
    # (QK -> bias -> exp -> AV) dependency chains are always in flight;
    # the DVE bias path writes to SBUF so the scores PSUM tile is freed as
    # soon as the stt has read it (shorter PSUM lifetime -> better overlap
    # with only 2 score buffers).
    for P in range(NP):
        isl = slice(P * 512, (P + 1) * 512)
        # per-panel bias tile (bf16), double-buffered against next panel
        biasP = bpool.tile([128, njt * 512], BF16, tag="biasP",
                           name=f"biasP_{P}")
        bview = biasP[:].rearrange("p (j x) -> p j x", j=njt)
        nc.sync.dma_start(bview[:], bias_ap[P])
        o = [[ops.tile([128, 512], F32, tag=f"o{pair}{t}",
                       name=f"o{pair}{t}_{P}")
              for t in range(2)] for pair in range(2)]
        # pair 1 walks the j-tiles at an offset so its PE/DVE bias slots
        # anti-align with pair 0's (accumulation order over j is free).
        stag = STAGGER if njt > 3 else 0
        exp_pair = EXP_PAIR and all(e == 'd' for e in eng)
        if exp_pair:
            stag = 0
        for jj in range(njt):
            zs2 = None
            if exp_pair:
                zs2 = spool.tile([128, 2048], F32, tag="zs2",
                                 name=f"zs2_{P}_{jj}")
            for pair in range(2):
                j = (jj + stag * pair) % njt
                jsl = slice(j * 128, (j + 1) * 128)
                bT = bview[:, j]
                use_pe = (eng[j] == 'p')
                zp = zps.tile([128, 1024], F32, tag="zp",
                              name=f"zp_{P}_{pair}_{j}")
                for t in range(2):
                    psl = slice(t * 64, (t + 1) * 64)
                    zsl = slice(t * 512, (t + 1) * 512)
                    nc.tensor.matmul(zp[:, zsl], lhsT=ktr[pair][psl, jsl],
                                     rhs=qtr[pair][psl, isl],
                                     start=True, stop=not use_pe,
                                     tile_position=((t * 64, 0)
                                                    if QK_TILE_POS else None))
                if use_pe:
                    for t in range(2):
                        h = 2 * pair + t
                        zsl = slice(t * 512, (t + 1) * 512)
                        nc.tensor.matmul(
                            zp[:, zsl],
                            lhsT=scaledI[:, h * 128:(h + 1) * 128],
                            rhs=bT, start=False, stop=True,
                            skip_group_check=True)
                    esrc = zp
                else:
                    zs = (zs2[:, pair * 1024:(pair + 1) * 1024] if exp_pair
                          else spool.tile([128, 1024], F32, tag="zs",
                                          name=f"zs_{P}_{pair}_{j}")[:])
                    for t in range(2):
                        h = 2 * pair + t
                        zsl = slice(t * 512, (t + 1) * 512)
                        nc.vector.scalar_tensor_tensor(
                            zs[:, zsl], in0=bT, scalar=n8tau[:, h:h + 1],
                            in1=zp[:, zsl], op0=Alu.mult, op1=Alu.add)
                    esrc = zs
                if exp_pair:
                    continue   # one fused exp + the AVs are emitted below
                et = epool.tile([128, 1024], F32R, tag="et",
                                name=f"et_{P}_{pair}_{j}")
                nc.scalar.activation(et[:], esrc[:], Act.Exp,
                                     bias=maskadd[:, j:j + 1], scale=0.125)
                for t in range(2):
                    h = 2 * pair + t
                    nc.tensor.matmul(
                        o[pair][t][0:65, :],
                        lhsT=vaug[h][:, j * 65:(j + 1) * 65],
                        rhs=et[:, t * 512:(t + 1) * 512],
                        start=(jj == 0), stop=(jj == njt - 1))
            if exp_pair:
                j = jj
                et = epool.tile([128, 2048], F32R, tag="et2",
                                name=f"et2_{P}_{jj}")
                nc.scalar.activation(et[:], zs2[:], Act.Exp,
                                     bias=maskadd[:, j:j + 1], scale=0.125)
                for pair in range(2):
                    for t in range(2):
                        h = 2 * pair + t
                        nc.tensor.matmul(
                            o[pair][t][0:65, :],
                            lhsT=vaug[h][:, j * 65:(j + 1) * 65],
                            rhs=et[:, (2 * pair + t) * 512:
                                    (2 * pair + t + 1) * 512],
                            start=(jj == 0), stop=(jj == njt - 1))

        # ---- drain: PSUM->SBUF copy + DMA (normalize on host) ----------
        for pair in range(2):
            for t in range(2):
                h = 2 * pair + t
                ob = dpool.tile([65, 512], F32, tag="ob", name=f"ob_{P}_{h}")
                nc.vector.tensor_copy(ob[:], o[pair][t][0:65, :])
                aux.dma_start(out_ap[h, P], ob[:])


_NC_CACHE = {}


def _get_nc(njt, sched=BIAS_SCHED, n_rep=1):
    key = (njt, sched, n_rep)
    if key not in _NC_CACHE:
        _NC_CACHE[key] = _build(njt, sched, n_rep)
    return _NC_CACHE[key]


def prepare_in_maps(q, k, v, mask, taus, attn_bias):
    """Host-side shard + gather. Returns (in_maps, njt, per-core key counts)."""
    q = np.asarray(q, dtype=np.float32)
    k = np.asarray(k, dtype=np.float32)
    v = np.asarray(v, dtype=np.float32)
    mask = np.asarray(mask).astype(bool)
    taus = np.asarray(taus, dtype=np.float32)
    attn_bias = np.asarray(attn_bias, dtype=np.float32)

    idxs = [np.nonzero(mask[b])[0] for b in range(B)]
    njs = [len(ix) for ix in idxs]
    njt = max(1, (max(njs) + 127) // 128)
    nj = njt * 128

    eye = np.eye(128, dtype=np.float32)

    per_batch = []
    for b in range(B):
        ix = idxs[b]
        n = njs[b]
        # gathered K^T [H,D,nj], V-aug [H,128,njt*65], bias [NP,128,njt,512]
        kg = np.zeros((H, D, nj), dtype=np.float32)
        kg[:, :, :n] = k[b][:, ix, :].swapaxes(1, 2)
        vg = np.zeros((H, nj, D), dtype=np.float32)
        vg[:, :n, :] = v[b][:, ix, :]
        va = np.ones((H, 128, njt, 65), dtype=np.float32)
        va[..., 0:64] = vg.reshape(H, njt, 128, D).transpose(0, 2, 1, 3)
        va = np.ascontiguousarray(va.reshape(H, 128, njt * 65))
        bg = np.zeros((nj, S), dtype=np.float32)
        bg[:n] = attn_bias[b][:, ix].T
        # [P, p, j, x] = biasT[j*128+p, P*512+x]
        bp = np.ascontiguousarray(
            bg.reshape(njt, 128, NP, 512).transpose(2, 1, 0, 3)
        ).astype(ml_dtypes.bfloat16)
        ma = np.zeros((128, njt), dtype=np.float32)
        g = np.arange(nj).reshape(njt, 128).T  # [p, j] global key index
        ma[g >= n] = -BIG
        per_batch.append((kg, va, bp, ma))

    in_maps = []
    for c in range(N_CORES):
        b = c // 4
        h0 = (c % 4) * 4
        kg, va, bp, ma = per_batch[b]
        t4 = taus[h0:h0 + HPC]
        si = np.einsum('pc,h->phc', eye, -8.0 * t4).reshape(128, HPC * 128)
        qk_np = (ml_dtypes.bfloat16 if QK_DT == mybir.dt.bfloat16
                 else np.float32)
        in_maps.append({
            "qt": np.ascontiguousarray(
                q[b, h0:h0 + HPC].swapaxes(1, 2)).reshape(2, 128, S)
                .astype(qk_np),
            "kt": np.ascontiguousarray(
                kg[h0:h0 + HPC]).reshape(2, 128, -1).astype(qk_np),
            "vp": va[h0:h0 + HPC],
            "n8tau": np.broadcast_to((-8.0 * t4)[None, :], (128, HPC)).copy(),
            "scaledI": si.astype(ml_dtypes.bfloat16),
            "maskadd": ma,
            "biasT": bp,
        })
    return in_maps, njt, njs


def kernel(q, k, v, mask, taus, attn_bias):
    in_maps, njt, _ = prepare_in_maps(q, k, v, mask, taus, attn_bias)
    nc = _get_nc(njt)
    res = bass_utils.run_bass_kernel_spmd(nc, in_maps,
                                          core_ids=list(range(N_CORES)))
    out = np.empty((B, H, S, D), dtype=np.float32)
    for c in range(N_CORES):
        b = c // 4
        h0 = (c % 4) * 4
        arr = res.results[c]["out"]          # [HPC, NP, 65, 512]
        norm = arr[:, :, 0:64, :] / arr[:, :, 64:65, :]
        # i = P*512 + col  ->  out[h, i, d]
        out[b, h0:h0 + HPC] = norm.transpose(0, 1, 3, 2).reshape(HPC, S, D)
    return out


if __name__ == "__main__":
    rng = np.random.default_rng(0)
    inputs = {
        "q": rng.standard_normal((B, H, S, D), dtype=np.float32),
        "k": rng.standard_normal((B, H, S, D), dtype=np.float32),
        "v": rng.standard_normal((B, H, S, D), dtype=np.float32),
        "mask": rng.random((B, S)) < 0.5,
        "taus": rng.random(H, dtype=np.float32),
        "attn_bias": rng.random((B, S, S), dtype=np.float32),
    }
    o = kernel(**inputs)
    print("out", o.shape, o.dtype, np.isfinite(o).all())
